# revision 1
# baseline (speedup 1.0000x reference)
"""Biased multi-head self-attention (B=4, N=1024, H=1024, 16 heads) on 8
Trainium2 NeuronCores.

Sharding: data-parallel over batch (4) x tensor-parallel over head-groups
(2 groups of 8 heads) = 8 cores. Core c handles batch c//2, head-group c%2.
Each core computes QKV projections for its 512 feature columns, biased
softmax attention for its 8 heads, and a partial output projection
(contracting its 512 feature rows of Wp). The two head-groups' partial
projections per batch are summed on the host, which also adds bp.

Device dataflow (per core):
  - xT [h, n] (host-transposed x) and weights [h, d] feed the PE directly:
    qT/kT land in [d, n] layout, v in [n, d] layout.
  - The attention-score scale 1/sqrt(64) is folded into Wq/bq on the host.
  - Scores are computed TRANSPOSED, sT[m, n] (lhsT=kT, rhs=qT, K=64),
    row-packed in pairs (heads 2h/2h+1 on PE row-groups 0-1/2-3).
  - The bias enters as et = exp(s) * exp(b): the host pre-exps the bias,
    es = exp(sT) on ACT (one [128,1024] 2-bank psum tile per head per
    m-chunk), et = es*eb on DVE (fp16 2x mode). PE density in the
    attention loop stays ~93% via PV+scores, keeping HAM at full clock.
  - Every K=128 contraction chain (v/qk/out projections) is split into
    two K=64 half-chains on disjoint PE row-groups feeding two psum
    banks; adjacent A/B matmuls stream concurrently, hiding the
    otherwise-serial LDWEIGHTS (~200ns/MM measured). The A+B sum is
    fused into the evacuation (ACT copies A with any per-partition
    bias; DVE adds B).
  - v is stored with an interleaved all-ones column per head (augmented
    Wv/bv), so each head's PV matmul yields [65, n]: rows 0..63 are
    unnormalized feats^T, row 64 the softmax denominator s[n].
  - 1/s via reciprocal_approx_fast on a packed [97,512] tile (rows
    0/32/64/96), broadcast across partitions via K=1 ones matmuls.
  - out_partial [n, 1024] = featsT.T @ Wp_slice, written fp16.
"""

import sys

for _p in ("/opt/trn_rl_repo", "/opt/pypackages"):
    if _p not in sys.path:
        sys.path.append(_p)

import numpy as np

import concourse.bass as bass
import concourse.bacc as bacc
import concourse.mybir as mybir
import concourse.tile as tile
from concourse.bass_utils import run_bass_kernel_spmd

P = 128
N = 1024          # sequence length
H = 1024          # model dim
B = 4
NH = 16
HS = 64
G = 2             # head groups (tensor parallel)
HL = NH // G      # heads per core = 8
DLOC = H // G     # feature cols per core = 512
DAUG = HL * (HS + 1)   # 520: v with interleaved ones column
HF = DAUG // 2    # 260
HC = H // P       # 8 contraction chunks over model dim
DC = DLOC // P    # 4 chunks over local feature dim
NB = N // 512     # 2 moving-dim blocks
NT = N // P       # 8 n tiles
MC = N // P       # 8 m chunks
SCALE = 1.0 / np.sqrt(HS)

F32 = mybir.dt.float32
F16 = mybir.dt.float16
Act = mybir.ActivationFunctionType

_PROG = None
SPLIT_K = False  # A/B K=64 half-chains crashed HW when applied everywhere
OUT_SPLIT = False  # split-K crashes HW even isolated to the out-projection


def _emit(nc, tc, io):
    xT, biasT, wq, wk, wv, wp, bq, bk, bv, ones, ident_d, onesb_d, out = io

    import contextlib

    ADD = mybir.AluOpType.add

    with contextlib.ExitStack() as ctx:
        consts = ctx.enter_context(tc.tile_pool(name="consts", bufs=1))
        qkv = ctx.enter_context(tc.tile_pool(name="qkv", bufs=1))
        stage1 = ctx.enter_context(tc.tile_pool(name="stage1", bufs=1))
        opool = ctx.enter_context(tc.tile_pool(name="opool", bufs=4))
        # PSUM: sps carves 2-bank [128,2,512] slots (x2 = 4 banks), fps
        # carves 1-bank 2KB slots (x4 = 4 banks) -> all 8 banks.
        sps = ctx.enter_context(tc.tile_pool(name="sps", bufs=2, space="PSUM"))
        fps = ctx.enter_context(tc.tile_pool(name="fps", bufs=4, space="PSUM"))
        eb_pool = ctx.enter_context(tc.tile_pool(name="eb", bufs=10))
        et_pool = ctx.enter_context(tc.tile_pool(name="et", bufs=6))
        es_pool = ctx.enter_context(tc.tile_pool(name="es", bufs=4))
        fu_pool = ctx.enter_context(tc.tile_pool(name="fu", bufs=8))
        nrm_pool = ctx.enter_context(tc.tile_pool(name="nrm", bufs=2))
        tmp_pool = ctx.enter_context(tc.tile_pool(name="tmp", bufs=3))

        ones_t = consts.tile([1, P], F16)
        nc.gpsimd.dma_start(out=ones_t, in_=ones)
        onesb = consts.tile([33, HS], F16)
        nc.gpsimd.dma_start(out=onesb, in_=onesb_d)
        bq_sb = consts.tile([P, DC], F32)
        nc.gpsimd.dma_start(out=bq_sb, in_=bq)
        bk_sb = consts.tile([P, DC], F32)
        nc.gpsimd.dma_start(out=bk_sb, in_=bk)
        bv_sb = consts.tile([1, DAUG], F16)
        nc.gpsimd.dma_start(out=bv_sb, in_=bv)
        ident = consts.tile([P, P], F16)
        nc.sync.dma_start(out=ident, in_=ident_d)
        wp_sb = consts.tile([P, DC, H], F16)

        qT_sb = qkv.tile([P, DC, N], F16)
        kT_sb = qkv.tile([P, DC, N], F16)
        v_sb = qkv.tile([P, MC, DAUG], F16)
        featsT_sb = qkv.tile([P, DC, N], F16)

        # input staging; descriptor generation spread across engine queues
        xT_m = [stage1.tile([P, HC // 2, N], F16, name=f"xTm{i}") for i in range(2)]
        wv_m = [
            stage1.tile([P, HC // 2, DAUG], F16, name=f"wvm{i}") for i in range(2)
        ]
        xr = xT.rearrange("(c p) n -> p c n", p=P)
        wr = wv.rearrange("(c p) d -> p c d", p=P)
        for i in range(2):
            nc.sync.dma_start(out=xT_m[i], in_=xr[:, 4 * i : 4 * i + 4])
            nc.gpsimd.dma_start(out=wv_m[i], in_=wr[:, 4 * i : 4 * i + 4])
        wq_m = stage1.tile([P, HC, DLOC], F16, name="wqm")
        nc.scalar.dma_start(out=wq_m, in_=wq.rearrange("(c p) d -> p c d", p=P))
        wk_m = stage1.tile([P, HC, DLOC], F16, name="wkm")
        nc.sync.dma_start(out=wk_m, in_=wk.rearrange("(c p) d -> p c d", p=P))
        nc.scalar.dma_start(out=wp_sb, in_=wp.rearrange("(c p) o -> p c o", p=P))
        xT_t = [xT_m[hc // 4][:, hc % 4] for hc in range(HC)]
        wv_t = [wv_m[hc // 4][:, hc % 4] for hc in range(HC)]
        wq_t = [wq_m[:, hc] for hc in range(HC)]
        wk_t = [wk_m[:, hc] for hc in range(HC)]

        # normalization scratch: s rows parked at partitions 0/32 of two
        # tiles (AP base partitions are restricted to {0,32,64})
        s2 = [nrm_pool.tile([33, 512], F32, name=f"s2_{i}") for i in range(2)]
        for t in s2:
            nc.vector.memset(t, 1.0)
        inv32 = [nrm_pool.tile([33, 512], F32, name=f"i32_{i}") for i in range(2)]
        inv2 = [nrm_pool.tile([33, 512], F16, name=f"i16_{i}") for i in range(2)]

        # Every K=128 contraction chain is split into two K=64 half-chains
        # on disjoint PE row-groups feeding two psum banks (A: rows 0-63,
        # B: rows 64-127). Adjacent A/B matmuls stream concurrently (the
        # LDWEIGHTS of one overlaps the stream of the other), which roughly
        # halves the effective per-matmul cost in this stack, where
        # LDWEIGHTS is otherwise never hidden (~200ns/MM overhead
        # measured). The A+B sum is fused into the evacuation: ACT copies
        # A (applying any per-partition bias for free), DVE adds B.

        # ---- v projection: nt-major, A/B half-chains per nt ----
        for nt in range(NT):
            psa = sps.tile([P, 2, 512], F32, tag="sps", name=f"vpa{nt}")
            psb = (
                sps.tile([P, 2, 512], F32, tag="sps", name=f"vpb{nt}")
                if SPLIT_K
                else None
            )
            for hc in range(HC):
                for half in range(2):
                    if SPLIT_K:
                        nc.tensor.matmul(
                            psa[:, half, :HF],
                            (xT_t[hc][:64, nt * P : (nt + 1) * P]),
                            (wv_t[hc][:64, half * HF : (half + 1) * HF]),
                            start=(hc == 0),
                            stop=False,
                        )
                        nc.tensor.matmul(
                            psb[:, half, :HF],
                            (xT_t[hc][64:, nt * P : (nt + 1) * P]),
                            (wv_t[hc][64:, half * HF : (half + 1) * HF]),
                            start=(hc == 0),
                            stop=(hc == HC - 1),
                        )
                    else:
                        nc.tensor.matmul(
                            psa[:, half, :HF],
                            (xT_t[hc][:, nt * P : (nt + 1) * P]),
                            (wv_t[hc][:, half * HF : (half + 1) * HF]),
                            start=(hc == 0),
                            stop=False,
                        )
            for half in range(2):
                nc.tensor.matmul(
                    psa[:, half, :HF],
                    (ones_t[:1, :P]),
                    (bv_sb[:1, half * HF : (half + 1) * HF]),
                    start=False,
                    stop=True,
                )
            if SPLIT_K:
                va = tmp_pool.tile([P, 2, HF], F16, tag="tmp2", name=f"va{nt}")
                nc.scalar.copy(va, psa[:, :, :HF])
                nc.vector.tensor_tensor(
                    out=v_sb[:, nt].rearrange("p (b x) -> p b x", b=2),
                    in0=va,
                    in1=psb[:, :, :HF],
                    op=ADD,
                )
            else:
                nc.scalar.copy(
                    v_sb[:, nt].rearrange("p (b x) -> p b x", b=2),
                    psa[:, :, :HF],
                )

        # ---- attention ----
        def emit_norm(state, tail=False):
            fu = state
            for t in range(2):
                nc.vector.reciprocal_approx_fast(out=inv32[t], in_=s2[t])
                nc.vector.tensor_copy(inv2[t], inv32[t])
            for h, nb in fu:
                po, ddc = HS * (h % 2), h // 2
                t, r = h % 2, 32 * nb
                if tail:
                    bp_t = fps.tile([HS, 512], F32, tag="fps", name=f"bps{h}{nb}")
                    b_ps = bp_t
                else:
                    bp_t = sps.tile(
                        [HS, 2, 512], F32, tag="sps", name=f"bps{h}{nb}"
                    )
                    b_ps = bp_t[:, 0]
                nc.tensor.matmul(
                    b_ps[:HS, :],
                    onesb[r : r + 1, :HS],
                    inv2[t][r : r + 1, :],
                    start=True,
                    stop=True,
                )
                nc.vector.tensor_mul(
                    out=featsT_sb[po : po + HS, ddc, nb * 512 : (nb + 1) * 512],
                    in0=fu[(h, nb)],
                    in1=b_ps[:HS, :],
                )

        pending_norm = None
        for hp in range(4):
            heads = (2 * hp, 2 * hp + 1)
            # q/k projections: per head pair, just before its attention
            for w_tiles, b_sb, dst in ((wq_t, bq_sb, qT_sb), (wk_t, bk_sb, kT_sb)):
                psa = sps.tile([P, 2, 512], F32, tag="sps", name=f"qka{hp}")
                psb = (
                    sps.tile([P, 2, 512], F32, tag="sps", name=f"qkb{hp}")
                    if SPLIT_K
                    else None
                )
                for hc in range(HC):
                    for nb in range(NB):
                        if SPLIT_K:
                            nc.tensor.matmul(
                                psa[:, nb],
                                (w_tiles[hc][:64, hp * P : (hp + 1) * P]),
                                (xT_t[hc][:64, nb * 512 : (nb + 1) * 512]),
                                start=(hc == 0),
                                stop=(hc == HC - 1),
                            )
                            nc.tensor.matmul(
                                psb[:, nb],
                                (w_tiles[hc][64:, hp * P : (hp + 1) * P]),
                                (xT_t[hc][64:, nb * 512 : (nb + 1) * 512]),
                                start=(hc == 0),
                                stop=(hc == HC - 1),
                            )
                        else:
                            nc.tensor.matmul(
                                psa[:, nb],
                                (w_tiles[hc][:, hp * P : (hp + 1) * P]),
                                (xT_t[hc][:, nb * 512 : (nb + 1) * 512]),
                                start=(hc == 0),
                                stop=(hc == HC - 1),
                            )
                if SPLIT_K:
                    qa = tmp_pool.tile([P, 2, 512], F16, tag="tmp", name=f"qa{hp}")
                    nc.scalar.activation(
                        out=qa,
                        in_=psa,
                        func=Act.Identity,
                        bias=b_sb[:, hp : hp + 1],
                    )
                    nc.vector.tensor_tensor(
                        out=dst[:, hp].rearrange("p (b x) -> p b x", b=2),
                        in0=qa,
                        in1=psb,
                        op=ADD,
                    )
                else:
                    nc.scalar.activation(
                        out=dst[:, hp].rearrange("p (b x) -> p b x", b=2),
                        in_=psa,
                        func=Act.Identity,
                        bias=b_sb[:, hp : hp + 1],
                    )
            bias_t = {}

            def load_bias(h, j):
                bt = eb_pool.tile([P, 2, N], F16, tag="eb", name=f"bt{h}_{j}")
                nc.gpsimd.dma_start(
                    out=bt,
                    in_=biasT[h].rearrange("(c p) n -> p c n", p=P)[
                        :, 2 * j : 2 * j + 2
                    ],
                )
                bias_t[h] = bt

            f_ps = {
                (h, nb): fps.tile([HS + 1, 512], F32, tag="fps", name=f"fps{h}_{nb}")
                for h in heads
                for nb in range(NB)
            }
            prev_pv = None
            for mc in range(MC):
                if mc % 2 == 0:
                    for h in heads:
                        load_bias(h, mc // 2)
                # K=64 scores, head pair row-packed (h0: rows 0-63, h1:
                # rows 64-127); emitted h-alternating so pairs overlap.
                # The bias enters as et = exp(s) * exp(b) on DVE; this
                # keeps the PE dense (PV dominates) without ident matmuls.
                h0, h1 = heads
                sp = {}
                for h in heads:
                    sp[h] = sps.tile(
                        [P, 2, 512], F32, tag="sps", name=f"sp{h}_{mc}"
                    )
                for nb in range(NB):
                    nc.tensor.matmul(
                        sp[h0][:, nb],
                        ident[:, :],
                        bias_t[h0][:, mc % 2, nb * 512 : (nb + 1) * 512],
                        start=True,
                        stop=False,
                    )
                for nb in range(NB):
                    for h in heads:
                        dpo = (h % 2) * HS
                        nc.tensor.matmul(
                            sp[h][:, nb],
                            kT_sb[dpo : dpo + HS, hp, mc * P : (mc + 1) * P],
                            qT_sb[dpo : dpo + HS, hp, nb * 512 : (nb + 1) * 512],
                            start=(h != h0),
                            stop=True,
                        )
                # PV for the previous m-chunk overlaps this chunk's exp
                if prev_pv is not None:
                    et_p, mc_p = prev_pv
                    for h in heads:
                        for nb in range(NB):
                            nc.tensor.matmul(
                                f_ps[(h, nb)],
                                v_sb[:, mc_p, (HS + 1) * h : (HS + 1) * (h + 1)],
                                et_p[h][:, nb * 512 : (nb + 1) * 512],
                                start=(mc_p == 0),
                                stop=(mc_p == MC - 1),
                            )
                et = {}
                et[h0] = et_pool.tile([P, N], F16, tag="et", name=f"et{h0}_{mc}")
                nc.scalar.activation(out=et[h0], in_=sp[h0], func=Act.Exp)
                es = es_pool.tile([P, N], F16, tag="es", name=f"es{h1}_{mc}")
                nc.scalar.activation(out=es, in_=sp[h1], func=Act.Exp)
                et[h1] = et_pool.tile([P, N], F16, tag="et", name=f"et{h1}_{mc}")
                nc.vector.tensor_mul(
                    out=et[h1], in0=es, in1=bias_t[h1][:, mc % 2]
                )
                prev_pv = (et, mc)
                if mc == 4 and pending_norm is not None:
                    emit_norm(pending_norm)
                    pending_norm = None
            et_p, mc_p = prev_pv
            for h in heads:
                for nb in range(NB):
                    nc.tensor.matmul(
                        f_ps[(h, nb)],
                        v_sb[:, mc_p, (HS + 1) * h : (HS + 1) * (h + 1)],
                        et_p[h][:, nb * 512 : (nb + 1) * 512],
                        start=False,
                        stop=True,
                    )
            # evacuate feats psum (ACT) + denominators (DVE) fast
            fu = {}
            for h, nb in f_ps:
                ft = fu_pool.tile([HS, 512], F32, tag="fu", name=f"fu{h}{nb}")
                nc.scalar.copy(ft, f_ps[(h, nb)][:HS, :])
                nc.vector.tensor_copy(
                    s2[h % 2][32 * nb : 32 * nb + 1, :],
                    f_ps[(h, nb)][HS : HS + 1, :],
                )
                fu[(h, nb)] = ft
            pending_norm = fu

        # ---- output projection (partial: contracts this core's 512 rows)
        # A-chain: dc0/dc1 half-chains; B-chain: dc2/dc3. dc3 depends on the
        # last pair's normalize, so B chains stay open across emit_norm while
        # A-chains + dc2 keep the PE busy; one fps slot pair is left free so
        # the norm broadcasts cannot deadlock.
        def proj_fin(nt, cb, psa, psb, dve=False):
            ot = opool.tile([P, 512], F16, tag="o", name=f"ot{nt}{cb}")
            if OUT_SPLIT:
                oa = opool.tile([P, 512], F16, tag="o", name=f"oa{nt}{cb}")
                nc.scalar.copy(oa, psa)
                nc.vector.tensor_tensor(out=ot, in0=oa, in1=psb, op=ADD)
            elif dve:
                nc.vector.tensor_copy(ot, psb)
            else:
                nc.scalar.copy(ot, psb)
            nc.sync.dma_start(
                out=out[nt * P : (nt + 1) * P, cb * 512 : (cb + 1) * 512], in_=ot
            )

        def proj_mm(nt, cb, ps, dcs, close=True):
            if OUT_SPLIT:
                for dc in dcs:
                    for hk in (0, 64):
                        nc.tensor.matmul(
                            ps,
                            (featsT_sb[hk : hk + 64, dc, nt * P : (nt + 1) * P]),
                            (wp_sb[hk : hk + 64, dc, cb * 512 : (cb + 1) * 512]),
                            start=(dc == dcs[0] and hk == 0),
                            stop=(close and dc == dcs[-1] and hk == 64),
                        )
            else:
                for dc in dcs:
                    nc.tensor.matmul(
                        ps,
                        (featsT_sb[:, dc, nt * P : (nt + 1) * P]),
                        (wp_sb[:, dc, cb * 512 : (cb + 1) * 512]),
                        start=(dc == dcs[0]),
                        stop=(close and dc == dcs[-1]),
                    )

        # early: nt0 via the two sps slots (A/B for both cb), nt1 cb0 via
        # two fps slots; two fps slots stay free for the norm broadcasts
        eps = {}
        if OUT_SPLIT:
            eps[(0, 0)] = (
                sps.tile([P, 2, 512], F32, tag="sps", name="opa0"),
                sps.tile([P, 2, 512], F32, tag="sps", name="opb0"),
            )
            eps[(1, 0)] = (
                fps.tile([P, 512], F32, tag="fps", name="opa1"),
                fps.tile([P, 512], F32, tag="fps", name="opb1"),
            )
            for (nt, cb), (psa, psb) in eps.items():
                if nt == 0:
                    for c2 in range(NB):
                        proj_mm(nt, c2, psa[:, c2], [0, 1])
                        proj_mm(nt, c2, psb[:, c2], [2], close=False)
                else:
                    proj_mm(nt, cb, psa, [0, 1])
                    proj_mm(nt, cb, psb, [2], close=False)
        else:
            ps0 = sps.tile([P, 2, 512], F32, tag="sps", name="opb0")
            eps[(0, 0)] = (None, ps0)
            eps[(1, 0)] = (None, fps.tile([P, 512], F32, tag="fps", name="opb1"))
            eps[(1, 1)] = (None, fps.tile([P, 512], F32, tag="fps", name="opb2"))
            for c2 in range(NB):
                proj_mm(0, c2, ps0[:, c2], [0, 1, 2], close=False)
            proj_mm(1, 0, eps[(1, 0)][1], [0, 1, 2], close=False)
            proj_mm(1, 1, eps[(1, 1)][1], [0, 1, 2], close=False)
        if pending_norm is not None:
            emit_norm(pending_norm, tail=True)
            pending_norm = None

        def proj_close(nt, cb, psa, psb):
            if OUT_SPLIT:
                for hk in (0, 64):
                    nc.tensor.matmul(
                        psb,
                        (featsT_sb[hk : hk + 64, 3, nt * P : (nt + 1) * P]),
                        (wp_sb[hk : hk + 64, 3, cb * 512 : (cb + 1) * 512]),
                        start=False,
                        stop=(hk == 64),
                    )
            else:
                nc.tensor.matmul(
                    psb,
                    (featsT_sb[:, 3, nt * P : (nt + 1) * P]),
                    (wp_sb[:, 3, cb * 512 : (cb + 1) * 512]),
                    start=False,
                    stop=True,
                )
            proj_fin(nt, cb, psa, psb)

        if OUT_SPLIT:
            psa, psb = eps[(0, 0)]
            for c2 in range(NB):
                proj_close(0, c2, psa[:, c2], psb[:, c2])
            proj_close(1, 0, *eps[(1, 0)])
            rest = [(1, 1)] + [(nt, cb) for nt in range(2, 8) for cb in range(2)]
            for j, (nt, cb) in enumerate(rest):
                if j % 2 == 0:
                    psa = sps.tile([P, 2, 512], F32, tag="sps", name=f"rpa{nt}{cb}")
                    psb = sps.tile([P, 2, 512], F32, tag="sps", name=f"rpb{nt}{cb}")
                    psa, psb = psa[:, 0], psb[:, 0]
                else:
                    psa = fps.tile([P, 512], F32, tag="fps", name=f"rpa{nt}{cb}")
                    psb = fps.tile([P, 512], F32, tag="fps", name=f"rpb{nt}{cb}")
                proj_mm(nt, cb, psa, [0, 1])
                proj_mm(nt, cb, psb, [2, 3])
                proj_fin(nt, cb, psa, psb)
        else:
            ps0 = eps[(0, 0)][1]
            for c2 in range(NB):
                proj_close(0, c2, None, ps0[:, c2])
            proj_close(1, 0, None, eps[(1, 0)][1])
            proj_close(1, 1, None, eps[(1, 1)][1])
            rest = [(nt, cb) for nt in range(2, 8) for cb in range(2)]
            for j, (nt, cb) in enumerate(rest):
                if j % 3 == 0:
                    ps = sps.tile([P, 2, 512], F32, tag="sps", name=f"rp{nt}{cb}")
                    ps = ps[:, 0]
                else:
                    ps = fps.tile([P, 512], F32, tag="fps", name=f"rp{nt}{cb}")
                proj_mm(nt, cb, ps, range(DC))
                proj_fin(nt, cb, None, ps, dve=bool(j % 2))


def build_program():
    nc = bacc.Bacc("TRN2", target_bir_lowering=False, debug=False, num_devices=8)
    xT = nc.dram_tensor("xT", [H, N], F16, kind="ExternalInput").ap()
    biasT = nc.dram_tensor("biasT", [HL, N, N], F16, kind="ExternalInput").ap()
    wq = nc.dram_tensor("wq", [H, DLOC], F16, kind="ExternalInput").ap()
    wk = nc.dram_tensor("wk", [H, DLOC], F16, kind="ExternalInput").ap()
    wv = nc.dram_tensor("wv", [H, DAUG], F16, kind="ExternalInput").ap()
    wp = nc.dram_tensor("wp", [DLOC, H], F16, kind="ExternalInput").ap()
    bq = nc.dram_tensor("bq", [P, DC], F32, kind="ExternalInput").ap()
    bk = nc.dram_tensor("bk", [P, DC], F32, kind="ExternalInput").ap()
    bv = nc.dram_tensor("bv", [1, DAUG], F16, kind="ExternalInput").ap()
    ones = nc.dram_tensor("ones", [1, P], F16, kind="ExternalInput").ap()
    ident_d = nc.dram_tensor("ident", [P, P], F16, kind="ExternalInput").ap()
    onesb_d = nc.dram_tensor("onesb", [33, HS], F16, kind="ExternalInput").ap()
    out = nc.dram_tensor("out", [N, H], F16, kind="ExternalOutput").ap()
    with tile.TileContext(nc) as tc:
        _emit(
            nc, tc,
            (xT, biasT, wq, wk, wv, wp, bq, bk, bv, ones, ident_d, onesb_d, out),
        )
    nc.compile()
    return nc


def get_program():
    global _PROG
    if _PROG is None:
        _PROG = build_program()
    return _PROG


def _mixed_bias(bias_hl):
    """[HL, N, N] -> transposed per-head bias; odd heads pre-exp'd (their
    bias enters multiplicatively as exp(b); even heads add via ident-MM,
    which doubles as PE filler that keeps HAM at full clock)."""
    bt = np.ascontiguousarray(bias_hl.transpose(0, 2, 1)).astype(np.float32)
    bt[1::2] = np.exp(bt[1::2])
    return bt.astype(np.float16)


def _onesb():
    s = np.zeros((33, HS), np.float16)
    s[0, :] = 1.0
    s[32, :] = 1.0
    return s


def make_in_maps(x, attn_bias, Wq, bq, Wk, bk, Wv, bv, Wp):
    """Host-side sharding: slice/transpose/augment per-core inputs."""
    f = np.float32
    x = np.asarray(x, f)
    attn_bias = np.asarray(attn_bias, f)
    wq_s = np.asarray(Wq, f) * f(SCALE)
    bq_s = np.asarray(bq, f) * f(SCALE)
    Wk, bk = np.asarray(Wk, f), np.asarray(bk, f)
    Wv, bv = np.asarray(Wv, f), np.asarray(bv, f)
    Wp = np.asarray(Wp, f)

    xTs = [np.ascontiguousarray(x[b].T).astype(np.float16) for b in range(B)]
    in_maps = []
    for c in range(8):
        b, g = divmod(c, 2)
        dsl = slice(DLOC * g, DLOC * (g + 1))
        wv_aug = np.zeros((H, DAUG), np.float16)
        bv_aug = np.zeros((1, DAUG), np.float16)
        for hl in range(HL):
            src = slice(DLOC * g + HS * hl, DLOC * g + HS * (hl + 1))
            dst = slice((HS + 1) * hl, (HS + 1) * hl + HS)
            wv_aug[:, dst] = Wv[:, src]
            bv_aug[0, dst] = bv[src]
            bv_aug[0, (HS + 1) * hl + HS] = 1.0
        in_maps.append(
            {
                "xT": xTs[b],
                "biasT": _mixed_bias(attn_bias[b, HL * g : HL * (g + 1)]),
                "wq": np.ascontiguousarray(wq_s[:, dsl]).astype(np.float16),
                "wk": np.ascontiguousarray(Wk[:, dsl]).astype(np.float16),
                "wv": wv_aug,
                "wp": np.ascontiguousarray(Wp[dsl, :]).astype(np.float16),
                "bq": np.ascontiguousarray(bq_s[dsl].reshape(DC, P).T),
                "bk": np.ascontiguousarray(bk[dsl].reshape(DC, P).T),
                "bv": bv_aug,
                "ones": np.ones((1, P), np.float16),
                "ident": np.eye(P, dtype=np.float16),
                "onesb": _onesb(),
            }
        )
    return in_maps


def _ensure_ntff_hook():
    """Register the axon NTFF profile hook if the image's antenv lacks it."""
    try:
        from antenv.axon_hooks import get_axon_ntff_profile_hook  # noqa: F401

        return
    except ImportError:
        pass
    import types

    import antenv
    from trn_agent_boot.trn_boot import _ntff_profile_via_ctypes

    mod = types.ModuleType("antenv.axon_hooks")
    box = {"h": None}
    mod.set_axon_ntff_profile_hook = lambda h: box.__setitem__("h", h)
    mod.get_axon_ntff_profile_hook = lambda: box["h"]
    sys.modules["antenv.axon_hooks"] = mod
    antenv.axon_hooks = mod
    hook = _ntff_profile_via_ctypes("/opt/axon/libaxon_pjrt.so")
    if hook is not None:
        mod.set_axon_ntff_profile_hook(hook)


def run_cores(in_maps, trace=False):
    nc = get_program()
    kwargs = {}
    if trace:
        _ensure_ntff_hook()
        kwargs = dict(trace=True, trace_cores=[0])
    return run_bass_kernel_spmd(nc, in_maps, core_ids=list(range(8)), **kwargs)


def kernel(x, attn_bias, Wq, bq, Wk, bk, Wv, bv, Wp, bp):
    in_maps = make_in_maps(x, attn_bias, Wq, bq, Wk, bk, Wv, bv, Wp)
    res = run_cores(in_maps)
    bp = np.asarray(bp, np.float32)
    out = np.empty((B, N, H), np.float32)
    for b in range(B):
        out[b] = (
            res.results[2 * b]["out"].astype(np.float32)
            + res.results[2 * b + 1]["out"].astype(np.float32)
            + bp
        )
    return out

